# revision 8
# baseline (speedup 1.0000x reference)
"""Additive (Bahdanau) attention on 8 Trainium2 NeuronCores.

Reference computation (per batch b):
    kp = key[:, b, :] @ Wk            (S, H)
    qp = query[:, b, :] @ Wq + bk+bq  (T, H)
    scores[t, s] = sum_h v[h] * tanh(qp[t, h] + kp[s, h])
    out[b] = softmax(scores, axis=s)  (T, S)

Sharding: batch B=8 across the 8 cores, one batch element per core.
No collectives needed.

Device layout: H on SBUF partitions (2 halves of 128).  The broadcast
add + tanh is fused into a single ScalarE ACTIVATE per (t, half) using
the per-partition bias operand: E = tanh(kp[h, s] + qp_t[h]).  The
reduction over h runs on the TensorEngine with v as the stationary
operand; softmax is fused via reduce_max(negate) + Exp(accum_out).
"""

import numpy as np

S, T, B = 1024, 64, 8
D, H = 512, 256
N_CORES = 8
PART = 128
ND = D // PART       # 4 contraction chunks
NH = H // PART       # 2 h halves
NS_F32 = S // 512    # matmul N<=512 fp32 chunks

_CACHE: dict = {}


def _build_nc():
    import concourse.bass as bass
    import concourse.tile as tile
    from concourse import bacc, mybir

    f32 = mybir.dt.float32
    AF = mybir.ActivationFunctionType

    nc = bacc.Bacc(
        "TRN2",
        target_bir_lowering=False,
        debug=False,
        num_devices=N_CORES,
    )

    keyT = nc.dram_tensor("keyT", [D, S], f32, kind="ExternalInput").ap()
    queryT = nc.dram_tensor("queryT", [D, T], f32, kind="ExternalInput").ap()
    wk = nc.dram_tensor("wk", [D, H], f32, kind="ExternalInput").ap()
    wq = nc.dram_tensor("wq", [D, H], f32, kind="ExternalInput").ap()
    bsum = nc.dram_tensor("bsum", [H], f32, kind="ExternalInput").ap()
    # vshift[h, p, j, m] = v[h*128+p] if j == m else 0.  Using column j as
    # the stationary operand routes score_t to PSUM partition 32g+j (= t).
    vshift = nc.dram_tensor("vshift", [NH, PART, 32, 32], f32, kind="ExternalInput").ap()
    out = nc.dram_tensor("out", [T, S], f32, kind="ExternalOutput").ap()

    with tile.TileContext(nc) as tc:
        with (
            tc.tile_pool(name="const", bufs=1) as cpool,
            tc.tile_pool(name="epool", bufs=3) as epool,
            tc.tile_pool(name="spool", bufs=1) as spool,
            tc.tile_pool(name="kp_ps", bufs=1, space="PSUM") as kp_pool,
            tc.tile_pool(name="ps_small", bufs=2, space="PSUM") as ps_small,
        ):
            # ---- load inputs ----
            keyT_sb = cpool.tile([PART, ND, S], f32)
            nc.sync.dma_start(keyT_sb[:], keyT.rearrange("(n p) s -> p n s", p=PART))
            queryT_sb = cpool.tile([PART, ND, T], f32)
            nc.sync.dma_start(queryT_sb[:], queryT.rearrange("(n p) t -> p n t", p=PART))
            wk_sb = cpool.tile([PART, ND, H], f32)
            nc.sync.dma_start(wk_sb[:], wk.rearrange("(n p) h -> p n h", p=PART))
            wq_sb = cpool.tile([PART, ND, H], f32)
            nc.sync.dma_start(wq_sb[:], wq.rearrange("(n p) h -> p n h", p=PART))
            bsum_sb = cpool.tile([PART, NH], f32)
            nc.sync.dma_start(bsum_sb[:], bsum.rearrange("(a p) -> p a", p=PART))
            vshift_sb = cpool.tile([PART, NH, 32, 32], f32)
            nc.sync.dma_start(vshift_sb[:], vshift.rearrange("h p j m -> p h j m"))

            # ---- projections: kp^T [h, s] (PSUM-resident), qp^T [h, t] ----
            kp_ps = [kp_pool.tile([PART, S], f32, tag=f"kp{h}", name=f"kp{h}") for h in range(NH)]
            for h in range(NH):
                for c in range(NS_F32):
                    for n in range(ND):
                        nc.tensor.matmul(
                            kp_ps[h][:, c * 512:(c + 1) * 512],
                            wk_sb[:, n, h * PART:(h + 1) * PART],
                            keyT_sb[:, n, c * 512:(c + 1) * 512],
                            start=(n == 0),
                            stop=(n == ND - 1),
                        )

            qp_sb = []
            for h in range(NH):
                qp_ps = ps_small.tile([PART, T], f32, tag="qp", name=f"qp_ps{h}")
                for n in range(ND):
                    nc.tensor.matmul(
                        qp_ps[:],
                        wq_sb[:, n, h * PART:(h + 1) * PART],
                        queryT_sb[:, n, :],
                        start=(n == 0),
                        stop=(n == ND - 1),
                    )
                q = cpool.tile([PART, T], f32, tag=f"qp_sb{h}", name=f"qp_sb{h}")
                # qp + (bk+bq), fused into the PSUM->SBUF copy
                nc.scalar.add(q[:], qp_ps[:], bsum_sb[:, h:h + 1])
                qp_sb.append(q)

            # ---- main loop: E = tanh(kp + qp_t); scores += v_h . E ----
            # scores_t lands on PSUM partition t via the shifted-v trick:
            # lhsT = vshift[:, h, t%32, :] has v in column t%32, so the
            # matmul writes v.E to row t of the 32-row col-group t//32
            # (zeros accumulate into the other 31 rows).
            scores_ps = ps_small.tile([T, S], f32, tag="qp")
            for t in range(T):
                g, j = divmod(t, 32)
                for h in range(NH):
                    e = epool.tile([PART, S], f32, tag="e", name=f"e_{t}_{h}")
                    nc.scalar.activation(
                        e[:], kp_ps[h][:], AF.Tanh, bias=qp_sb[h][:, t:t + 1]
                    )
                    for c in range(NS_F32):
                        nc.tensor.matmul(
                            scores_ps[32 * g:32 * (g + 1), c * 512:(c + 1) * 512],
                            vshift_sb[:, h, j, :],
                            e[:, c * 512:(c + 1) * 512],
                            start=(j == 0 and h == 0),
                            stop=(j == 31 and h == NH - 1),
                        )

            # ---- softmax over s (free axis), t on partitions ----
            import concourse.mybir as mybir_mod

            negmax = spool.tile([T, 1], f32)
            nc.vector.tensor_reduce(
                negmax[:], scores_ps[:], axis=mybir_mod.AxisListType.X,
                op=mybir_mod.AluOpType.max, negate=True,
            )
            p_sb = spool.tile([T, S], f32)
            ssum = spool.tile([T, 1], f32)
            nc.scalar.activation(
                p_sb[:], scores_ps[:], AF.Exp, bias=negmax[:], accum_out=ssum[:]
            )
            rinv = spool.tile([T, 1], f32)
            nc.vector.reciprocal(rinv[:], ssum[:])
            out_sb = spool.tile([T, S], f32)
            nc.vector.tensor_scalar_mul(out_sb[:], p_sb[:], rinv[:])
            nc.sync.dma_start(out[:], out_sb[:])

    nc.compile()
    return nc


def _get_nc():
    if "nc" not in _CACHE:
        _CACHE["nc"] = _build_nc()
    return _CACHE["nc"]


def _in_maps(key, query, Wk, bk, Wq, bq, v):
    key = np.asarray(key, dtype=np.float32)
    query = np.asarray(query, dtype=np.float32)
    keyT = np.ascontiguousarray(key.transpose(1, 2, 0))      # (B, D, S)
    queryT = np.ascontiguousarray(query.transpose(1, 2, 0))  # (B, D, T)
    wk = np.ascontiguousarray(np.asarray(Wk, dtype=np.float32))
    wq = np.ascontiguousarray(np.asarray(Wq, dtype=np.float32))
    bsum = np.asarray(bk, dtype=np.float32) + np.asarray(bq, dtype=np.float32)
    vv = np.asarray(v, dtype=np.float32)
    vshift = np.zeros((NH, PART, 32, 32), dtype=np.float32)
    for h in range(NH):
        for j in range(32):
            vshift[h, :, j, j] = vv[h * PART:(h + 1) * PART]
    return [
        {
            "keyT": keyT[b], "queryT": queryT[b],
            "wk": wk, "wq": wq, "bsum": bsum, "vshift": vshift,
        }
        for b in range(N_CORES)
    ]


def kernel(key, query, Wk, bk, Wq, bq, v):
    from concourse.bass_utils import run_bass_kernel_spmd

    nc = _get_nc()
    in_maps = _in_maps(key, query, Wk, bk, Wq, bq, v)
    res = run_bass_kernel_spmd(nc, in_maps, core_ids=list(range(N_CORES)))
    return np.stack([res.results[b]["out"] for b in range(N_CORES)])


def _ensure_ntff_hook():
    """Provide antenv.axon_hooks (absent in this image) so that
    run_bass_kernel_spmd(trace=True) can drive NTFF profiling via the
    libaxon_pjrt.so C ABI directly."""
    import sys
    import types
    import ctypes
    import contextlib

    try:
        from antenv.axon_hooks import get_axon_ntff_profile_hook  # noqa: F401
        return
    except ImportError:
        pass

    import antenv

    holder = {}
    mod = types.ModuleType("antenv.axon_hooks")
    mod.set_axon_ntff_profile_hook = lambda h: holder.__setitem__("h", h)
    mod.get_axon_ntff_profile_hook = lambda: holder.get("h")
    sys.modules["antenv.axon_hooks"] = mod
    antenv.axon_hooks = mod

    so_path = "/opt/axon/libaxon_pjrt.so"
    lib = ctypes.CDLL(so_path)
    if not hasattr(lib, "axon_start_nrt_profile"):
        return
    lib.axon_start_nrt_profile.argtypes = [
        ctypes.POINTER(ctypes.c_int64),
        ctypes.c_size_t,
    ]
    lib.axon_start_nrt_profile.restype = ctypes.c_int64
    lib.axon_stop_nrt_profile.argtypes = [ctypes.c_char_p]
    lib.axon_stop_nrt_profile.restype = ctypes.c_int64

    @contextlib.contextmanager
    def _hook(output_dir, device_ids):
        import jax

        jax.devices()
        if device_ids:
            ids = (ctypes.c_int64 * len(device_ids))(*device_ids)
            rc = lib.axon_start_nrt_profile(ids, len(device_ids))
        else:
            rc = lib.axon_start_nrt_profile(None, 0)
        if rc != 0:
            raise RuntimeError(f"axon_start_nrt_profile rc={rc}")
        try:
            yield
        finally:
            n = lib.axon_stop_nrt_profile(str(output_dir).encode())
            print(f"ntff profile: {n} file(s) written to {output_dir}")

    mod.set_axon_ntff_profile_hook(_hook)


def kernel_traced(key, query, Wk, bk, Wq, bq, v):
    """Same as kernel() but captures the neuron profile; returns
    (output, exec_time_ns, trace_path)."""
    from concourse.bass_utils import run_bass_kernel_spmd

    _ensure_ntff_hook()
    nc = _get_nc()
    in_maps = _in_maps(key, query, Wk, bk, Wq, bq, v)
    res = run_bass_kernel_spmd(
        nc, in_maps, core_ids=list(range(N_CORES)), trace=True
    )
    outp = np.stack([res.results[b]["out"] for b in range(N_CORES)])
    trace_path = None
    if res.instructions_and_trace is not None:
        trace_path = res.instructions_and_trace[1]
    return outp, res.exec_time_ns, trace_path


# revision 9
# speedup vs baseline: 1.5272x; 1.5272x over previous
"""Additive (Bahdanau) attention on 8 Trainium2 NeuronCores.

Reference computation (per batch b):
    kp = key[:, b, :] @ Wk            (S, H)
    qp = query[:, b, :] @ Wq + bk+bq  (T, H)
    scores[t, s] = sum_h v[h] * tanh(qp[t, h] + kp[s, h])
    out[b] = softmax(scores, axis=s)  (T, S)

Sharding: batch B=8 across the 8 cores, one batch element per core.
No collectives needed.

Device layout: H on SBUF partitions (2 halves of 128).  The broadcast
add + tanh is fused into a single ScalarE ACTIVATE per (t, half) using
the per-partition bias operand: E = tanh(kp[h, s] + qp_t[h]).  The
reduction over h runs on the TensorEngine with v as the stationary
operand; softmax is fused via reduce_max(negate) + Exp(accum_out).
"""

import numpy as np

S, T, B = 1024, 64, 8
D, H = 512, 256
N_CORES = 8
PART = 128
ND = D // PART       # 4 contraction chunks
NH = H // PART       # 2 h halves
NS_F32 = S // 512    # matmul N<=512 fp32 chunks

_CACHE: dict = {}


def _build_nc():
    import concourse.bass as bass
    import concourse.tile as tile
    from concourse import bacc, mybir

    f32 = mybir.dt.float32
    AF = mybir.ActivationFunctionType

    nc = bacc.Bacc(
        "TRN2",
        target_bir_lowering=False,
        debug=False,
        num_devices=N_CORES,
    )

    keyT = nc.dram_tensor("keyT", [D, S], f32, kind="ExternalInput").ap()
    queryT = nc.dram_tensor("queryT", [D, T], f32, kind="ExternalInput").ap()
    wk = nc.dram_tensor("wk", [D, H], f32, kind="ExternalInput").ap()
    wq = nc.dram_tensor("wq", [D, H], f32, kind="ExternalInput").ap()
    bsum = nc.dram_tensor("bsum", [H], f32, kind="ExternalInput").ap()
    # vshift[h, p, j, m] = v[h*128+p] if j == m else 0.  Using column j as
    # the stationary operand routes score_t to PSUM partition 32g+j (= t).
    bf16 = mybir.dt.bfloat16
    vshift = nc.dram_tensor("vshift", [NH, PART, 32, 32], bf16, kind="ExternalInput").ap()
    out = nc.dram_tensor("out", [T, S], f32, kind="ExternalOutput").ap()

    with tile.TileContext(nc) as tc:
        with (
            tc.tile_pool(name="const", bufs=1) as cpool,
            tc.tile_pool(name="epool", bufs=3) as epool,
            tc.tile_pool(name="spool", bufs=1) as spool,
            tc.tile_pool(name="kp_ps", bufs=1, space="PSUM") as kp_pool,
            tc.tile_pool(name="ps_small", bufs=2, space="PSUM") as ps_small,
        ):
            # ---- load inputs ----
            keyT_sb = cpool.tile([PART, ND, S], f32)
            nc.sync.dma_start(keyT_sb[:], keyT.rearrange("(n p) s -> p n s", p=PART))
            queryT_sb = cpool.tile([PART, ND, T], f32)
            nc.sync.dma_start(queryT_sb[:], queryT.rearrange("(n p) t -> p n t", p=PART))
            wk_sb = cpool.tile([PART, ND, H], f32)
            nc.sync.dma_start(wk_sb[:], wk.rearrange("(n p) h -> p n h", p=PART))
            wq_sb = cpool.tile([PART, ND, H], f32)
            nc.sync.dma_start(wq_sb[:], wq.rearrange("(n p) h -> p n h", p=PART))
            bsum_sb = cpool.tile([PART, NH], f32)
            nc.sync.dma_start(bsum_sb[:], bsum.rearrange("(a p) -> p a", p=PART))
            vshift_sb = cpool.tile([PART, NH, 32, 32], bf16)
            nc.sync.dma_start(vshift_sb[:], vshift.rearrange("h p j m -> p h j m"))

            # ---- projections: kp^T [h, s] (PSUM-resident), qp^T [h, t] ----
            kp_ps = [kp_pool.tile([PART, S], f32, tag=f"kp{h}", name=f"kp{h}") for h in range(NH)]
            for h in range(NH):
                for c in range(NS_F32):
                    for n in range(ND):
                        nc.tensor.matmul(
                            kp_ps[h][:, c * 512:(c + 1) * 512],
                            wk_sb[:, n, h * PART:(h + 1) * PART],
                            keyT_sb[:, n, c * 512:(c + 1) * 512],
                            start=(n == 0),
                            stop=(n == ND - 1),
                        )

            qp_sb = []
            for h in range(NH):
                qp_ps = ps_small.tile([PART, T], f32, tag="qp", name=f"qp_ps{h}")
                for n in range(ND):
                    nc.tensor.matmul(
                        qp_ps[:],
                        wq_sb[:, n, h * PART:(h + 1) * PART],
                        queryT_sb[:, n, :],
                        start=(n == 0),
                        stop=(n == ND - 1),
                    )
                q = cpool.tile([PART, T], f32, tag=f"qp_sb{h}", name=f"qp_sb{h}")
                # qp + (bk+bq), fused into the PSUM->SBUF copy
                nc.scalar.add(q[:], qp_ps[:], bsum_sb[:, h:h + 1])
                qp_sb.append(q)

            # ---- main loop: E = tanh(kp + qp_t); scores += v_h . E ----
            # scores_t lands on PSUM partition t via the shifted-v trick:
            # lhsT = vshift[:, h, t%32, :] has v in column t%32, so the
            # matmul writes v.E to row t of the 32-row col-group t//32
            # (zeros accumulate into the other 31 rows).
            scores_ps = ps_small.tile([T, S], f32, tag="qp")
            for t in range(T):
                g, j = divmod(t, 32)
                for h in range(NH):
                    e = epool.tile([PART, S], bf16, tag="e", name=f"e_{t}_{h}")
                    nc.scalar.activation(
                        e[:], kp_ps[h][:], AF.Tanh, bias=qp_sb[h][:, t:t + 1]
                    )
                    for c in range(NS_F32):
                        nc.tensor.matmul(
                            scores_ps[32 * g:32 * (g + 1), c * 512:(c + 1) * 512],
                            vshift_sb[:, h, j, :],
                            e[:, c * 512:(c + 1) * 512],
                            start=(j == 0 and h == 0),
                            stop=(j == 31 and h == NH - 1),
                        )

            # ---- softmax over s (free axis), t on partitions ----
            import concourse.mybir as mybir_mod

            negmax = spool.tile([T, 1], f32)
            nc.vector.tensor_reduce(
                negmax[:], scores_ps[:], axis=mybir_mod.AxisListType.X,
                op=mybir_mod.AluOpType.max, negate=True,
            )
            p_sb = spool.tile([T, S], f32)
            ssum = spool.tile([T, 1], f32)
            nc.scalar.activation(
                p_sb[:], scores_ps[:], AF.Exp, bias=negmax[:], accum_out=ssum[:]
            )
            rinv = spool.tile([T, 1], f32)
            nc.vector.reciprocal(rinv[:], ssum[:])
            out_sb = spool.tile([T, S], f32)
            nc.vector.tensor_scalar_mul(out_sb[:], p_sb[:], rinv[:])
            nc.sync.dma_start(out[:], out_sb[:])

    nc.compile()
    return nc


def _get_nc():
    if "nc" not in _CACHE:
        _CACHE["nc"] = _build_nc()
    return _CACHE["nc"]


def _in_maps(key, query, Wk, bk, Wq, bq, v):
    key = np.asarray(key, dtype=np.float32)
    query = np.asarray(query, dtype=np.float32)
    keyT = np.ascontiguousarray(key.transpose(1, 2, 0))      # (B, D, S)
    queryT = np.ascontiguousarray(query.transpose(1, 2, 0))  # (B, D, T)
    wk = np.ascontiguousarray(np.asarray(Wk, dtype=np.float32))
    wq = np.ascontiguousarray(np.asarray(Wq, dtype=np.float32))
    bsum = np.asarray(bk, dtype=np.float32) + np.asarray(bq, dtype=np.float32)
    vv = np.asarray(v, dtype=np.float32)
    import ml_dtypes
    vshift = np.zeros((NH, PART, 32, 32), dtype=ml_dtypes.bfloat16)
    for h in range(NH):
        for j in range(32):
            vshift[h, :, j, j] = vv[h * PART:(h + 1) * PART]
    return [
        {
            "keyT": keyT[b], "queryT": queryT[b],
            "wk": wk, "wq": wq, "bsum": bsum, "vshift": vshift,
        }
        for b in range(N_CORES)
    ]


def kernel(key, query, Wk, bk, Wq, bq, v):
    from concourse.bass_utils import run_bass_kernel_spmd

    nc = _get_nc()
    in_maps = _in_maps(key, query, Wk, bk, Wq, bq, v)
    res = run_bass_kernel_spmd(nc, in_maps, core_ids=list(range(N_CORES)))
    return np.stack([res.results[b]["out"] for b in range(N_CORES)])


def _ensure_ntff_hook():
    """Provide antenv.axon_hooks (absent in this image) so that
    run_bass_kernel_spmd(trace=True) can drive NTFF profiling via the
    libaxon_pjrt.so C ABI directly."""
    import sys
    import types
    import ctypes
    import contextlib

    try:
        from antenv.axon_hooks import get_axon_ntff_profile_hook  # noqa: F401
        return
    except ImportError:
        pass

    import antenv

    holder = {}
    mod = types.ModuleType("antenv.axon_hooks")
    mod.set_axon_ntff_profile_hook = lambda h: holder.__setitem__("h", h)
    mod.get_axon_ntff_profile_hook = lambda: holder.get("h")
    sys.modules["antenv.axon_hooks"] = mod
    antenv.axon_hooks = mod

    so_path = "/opt/axon/libaxon_pjrt.so"
    lib = ctypes.CDLL(so_path)
    if not hasattr(lib, "axon_start_nrt_profile"):
        return
    lib.axon_start_nrt_profile.argtypes = [
        ctypes.POINTER(ctypes.c_int64),
        ctypes.c_size_t,
    ]
    lib.axon_start_nrt_profile.restype = ctypes.c_int64
    lib.axon_stop_nrt_profile.argtypes = [ctypes.c_char_p]
    lib.axon_stop_nrt_profile.restype = ctypes.c_int64

    @contextlib.contextmanager
    def _hook(output_dir, device_ids):
        import jax

        jax.devices()
        if device_ids:
            ids = (ctypes.c_int64 * len(device_ids))(*device_ids)
            rc = lib.axon_start_nrt_profile(ids, len(device_ids))
        else:
            rc = lib.axon_start_nrt_profile(None, 0)
        if rc != 0:
            raise RuntimeError(f"axon_start_nrt_profile rc={rc}")
        try:
            yield
        finally:
            n = lib.axon_stop_nrt_profile(str(output_dir).encode())
            print(f"ntff profile: {n} file(s) written to {output_dir}")

    mod.set_axon_ntff_profile_hook(_hook)


def kernel_traced(key, query, Wk, bk, Wq, bq, v):
    """Same as kernel() but captures the neuron profile; returns
    (output, exec_time_ns, trace_path)."""
    from concourse.bass_utils import run_bass_kernel_spmd

    _ensure_ntff_hook()
    nc = _get_nc()
    in_maps = _in_maps(key, query, Wk, bk, Wq, bq, v)
    res = run_bass_kernel_spmd(
        nc, in_maps, core_ids=list(range(N_CORES)), trace=True
    )
    outp = np.stack([res.results[b]["out"] for b in range(N_CORES)])
    trace_path = None
    if res.instructions_and_trace is not None:
        trace_path = res.instructions_and_trace[1]
    return outp, res.exec_time_ns, trace_path
